# revision 3
# baseline (speedup 1.0000x reference)
"""BiRealLinear Trainium2 kernel (w-stationary fp8 DoubleRow, f16 out).

Computes out = binact(x) @ quant_weight(w).T for
  x [4, 2048, 4096] f32, w [4096, 4096] f32  ->  out [4, 2048, 4096] f32

Forward semantics (STE parts drop out in forward):
  binact(x)       = sign(x)                      in {-1, 0, +1}
  quant_weight(w) = mean(|w|, axis=1) * sign(w)  per-output-row scale

So out[t, o] = scale[o] * sum_i sign(x[t,i]) * sign(w[o,i]).

8 cores in a 4 (token) x 2 (out-feature) grid; each core does a
[2048 x 4096 x 2048] sign-matmul. Host supplies both operands
transposed (contraction dim i outermost): w as bf16 k-tiles (bf16 is
sign-exact and loses <1e-4 on mean|w|), x as fp8e5 k-tiles (sign-exact
except |x| < 2^-17, ~6e-6 of entries). On-device per core:
  - plain contiguous DMA loads of the k-tiles,
  - ACT sign -> fp8e4 (+/-1 exact) pair-tiles,
  - |w| = w * sign(w) (exact) summed on DVE into 4 f16 accumulators,
    reduced by a short tree + 16 one-column PE matmuls against a ones
    vector -> scale COLUMN [128, 16] (o on partitions),
  - PE matmul fp8 DoubleRow, W STATIONARY: out psum = [128 o, 512 t],
    so the per-o scale is a per-partition scalar downstream,
  - contraction split in 2 chunks of 2048 and tokens in 2 halves so
    matmuls start before all operands are loaded (exact f16 SBUF
    accumulator between chunks; integer sums <= 2048 per chunk),
  - drain: chunk0 psum -> f16 acc (DVE copy); chunk1: td = psum + acc
    (DVE add), out_f16 = ACT Copy(td * scale_col) -> DMA (gpsimd queue).
Output DRAM is [o, t] f16 per core; host transposes + upcasts to f32.
"""

import sys

import numpy as np

try:
    import concourse.bacc as bacc  # noqa: F401
except ImportError:
    sys.path.insert(0, "/opt/trn_rl_repo")

import ml_dtypes

import concourse.bacc as bacc
import concourse.mybir as mybir
import concourse.tile as tile
from concourse.bass_utils import run_bass_kernel_spmd

dt = mybir.dt

# ---- problem geometry (hardcoded; full input is [8192, 4096] x [4096, 4096])
B, S, I_FULL, O_FULL = 4, 2048, 4096, 4096
T_FULL = B * S                      # 8192 tokens
T_GRID, O_GRID = 4, 2               # core grid: 4 token shards x 2 out shards
T_SH = T_FULL // T_GRID             # 2048 tokens per core
O_SH = O_FULL // O_GRID             # 2048 out features per core

P = 128                             # partitions
NK = I_FULL // P                    # 32 k-tiles
NKP = NK // 2                       # 16 k-pairs (DoubleRow contracts 2 tiles)
NCH = 2                             # contraction chunks (k-pairs 0-7 / 8-15)
KPC = NKP // NCH                    # 8 k-pairs per chunk
NTH = 2                             # token halves
TH = T_SH // NTH                    # 1024 tokens per half
NTP = TH // 512                     # 2 token panels (512) per half
NOB = O_SH // P                     # 16 o-blocks of 128
NWA = 4                             # |w| k-reduce accumulators


def build_nc():
    nc = bacc.Bacc("TRN2", target_bir_lowering=False, debug=False, num_devices=8)
    x = nc.dram_tensor("x", [NK, P, T_SH], dt.float8e5, kind="ExternalInput")
    w = nc.dram_tensor("w", [NK, P, O_SH], dt.bfloat16, kind="ExternalInput")
    out = nc.dram_tensor("out", [O_SH, T_SH], dt.float16, kind="ExternalOutput")

    with tile.TileContext(nc) as tc:
        with (
            tc.tile_pool(name="single", bufs=1) as sb,
            tc.tile_pool(name="wstg", bufs=3) as wstg,
            tc.tile_pool(name="xstg", bufs=4) as xstg,
            tc.tile_pool(name="wab", bufs=4) as wabp,
            tc.tile_pool(name="wacc", bufs=NWA) as waccp,
            tc.tile_pool(name="wp", bufs=NKP) as wpp,
            tc.tile_pool(name="xp", bufs=20) as xpp,
            tc.tile_pool(name="acc", bufs=36) as accp,
            tc.tile_pool(name="td", bufs=4) as tdp,
            tc.tile_pool(name="og", bufs=4) as ogp,
            tc.tile_pool(name="ps", bufs=6, space="PSUM") as psp,
            tc.tile_pool(name="pss", bufs=1, space="PSUM") as pssp,
        ):
            ones_col = sb.tile([P, 1], dt.float16)      # k-reduce ones
            nc.vector.memset(ones_col[:], 1.0)
            s_cols = sb.tile([P, NOB], dt.float32)      # scale columns
            pscol = pssp.tile([P, NOB], dt.float32, tag="pss", name="pscol")

            WP = [None] * NKP                 # fp8 w pair-tiles, all resident
            XP = {}                           # (th, kp) -> fp8 x pair-tile
            WACC = [None] * NWA               # f16 |w| accumulators

            def load_w_pair(kp):
                wp_t = wpp.tile([P, 2, O_SH], dt.float8e4, tag="wp",
                                name=f"wp_{kp}")
                for j in range(2):
                    k = 2 * kp + j
                    wtb = wstg.tile([P, O_SH], dt.bfloat16, tag="wstg",
                                    name=f"wtb_{k}")
                    nc.sync.dma_start(wtb[:], w[k])
                    nc.scalar.sign(wp_t[:, j, :], wtb[:])
                    wab = wabp.tile([P, O_SH], dt.float16, tag="wab",
                                    name=f"wab_{k}")
                    nc.vector.tensor_mul(wab[:], wtb[:], wp_t[:, j, :])
                    a = k % NWA
                    if WACC[a] is None:
                        wa = waccp.tile([P, O_SH], dt.float16, tag="wacc",
                                        name=f"wacc_{a}")
                        nc.vector.tensor_copy(wa[:], wab[:])
                        WACC[a] = wa
                    else:
                        nc.vector.tensor_add(WACC[a][:], wab[:], WACC[a][:])
                WP[kp] = wp_t

            def load_x_pair(th, kp):
                xp_t = xpp.tile([P, 2, TH], dt.float8e4, tag="xp",
                                name=f"xp_{th}_{kp}")
                for j in range(2):
                    k = 2 * kp + j
                    xtb = xstg.tile([P, TH], dt.float8e5, tag="xstg",
                                    name=f"xtb_{th}_{k}")
                    nc.sync.dma_start(xtb[:], x[k][:, th * TH:(th + 1) * TH])
                    nc.scalar.sign(xp_t[:, j, :], xtb[:])
                XP[(th, kp)] = xp_t

            def compute_scale():
                # tree-reduce the 4 accumulators, then 16 one-column PE
                # matmuls vs ones -> scale as columns [128, 16] in psum.
                t0 = wabp.tile([P, O_SH], dt.float16, tag="wab", name="wt0")
                nc.vector.tensor_add(t0[:], WACC[0][:], WACC[1][:])
                t1 = wabp.tile([P, O_SH], dt.float16, tag="wab", name="wt1")
                nc.vector.tensor_add(t1[:], WACC[2][:], WACC[3][:])
                nc.vector.tensor_add(t0[:], t0[:], t1[:])
                for ob in range(NOB):
                    nc.tensor.matmul(
                        pscol[:, ob:ob + 1],
                        lhsT=t0[:, ob * P:(ob + 1) * P],
                        rhs=ones_col[:],
                        start=True, stop=True,
                        skip_group_check=True,
                    )
                nc.vector.tensor_scalar_mul(s_cols[:], pscol[:], 1.0 / I_FULL)

            def mm_chunk(th, c, accs):
                first, last = (c == 0), (c == NCH - 1)
                for ob in range(NOB):
                    for tp in range(NTP):
                        u = ob * NTP + tp
                        pst = psp.tile([P, 512], dt.float32, tag="ps",
                                       name=f"ps_{th}_{c}_{u}")
                        for kpl in range(KPC):
                            kp = c * KPC + kpl
                            nc.tensor.matmul(
                                pst[:],
                                lhsT=WP[kp][:, :, ob * P:(ob + 1) * P],
                                rhs=XP[(th, kp)][:, :, tp * 512:(tp + 1) * 512],
                                start=(kpl == 0), stop=(kpl == KPC - 1),
                                perf_mode=mybir.MatmulPerfMode.DoubleRow,
                            )
                        if first:
                            acc = accp.tile([P, 512], dt.float16, tag="acc",
                                            name=f"acc_{th}_{u}")
                            accs[u] = acc
                            nc.vector.tensor_copy(acc[:], pst[:])
                        if last:
                            td = tdp.tile([P, 512], dt.float16, tag="td",
                                          name=f"td_{th}_{u}")
                            nc.vector.tensor_add(td[:], pst[:], accs[u][:])
                            og = ogp.tile([P, 512], dt.float16, tag="og",
                                          name=f"og_{th}_{u}")
                            nc.scalar.activation(
                                og[:], td[:],
                                mybir.ActivationFunctionType.Copy,
                                scale=s_cols[:, ob:ob + 1])
                            nc.gpsimd.dma_start(
                                out[ob * P:(ob + 1) * P,
                                    (th * NTP + tp) * 512:
                                    (th * NTP + tp + 1) * 512],
                                og[:])

            # ---- program order: loads/signs feed the PE just ahead of use.
            for kp in range(KPC):             # chunk 0 loads + signs
                load_w_pair(kp)
                load_x_pair(0, kp)
            accs_a = [None] * (NOB * NTP)
            mm_chunk(0, 0, accs_a)
            for kp in range(KPC, NKP):        # chunk 1 loads + signs
                load_w_pair(kp)
                load_x_pair(0, kp)
            compute_scale()
            for kp in range(KPC):             # prefetch x th1 chunk0
                load_x_pair(1, kp)
            mm_chunk(0, 1, accs_a)
            accs_b = [None] * (NOB * NTP)
            mm_chunk(1, 0, accs_b)
            for kp in range(KPC, NKP):        # prefetch x th1 chunk1
                load_x_pair(1, kp)
            mm_chunk(1, 1, accs_b)

    nc.compile()
    return nc


_NC_CACHE = None


def _get_nc():
    global _NC_CACHE
    if _NC_CACHE is None:
        _NC_CACHE = build_nc()
    return _NC_CACHE


def make_in_maps(x, weight):
    """Host-side shard + layout prep: per-core transposed k-tiles."""
    bf16 = ml_dtypes.bfloat16
    f8e5 = ml_dtypes.float8_e5m2
    x = np.asarray(x, dtype=np.float32).reshape(T_FULL, I_FULL)
    weight = np.asarray(weight, dtype=np.float32)
    xts = []
    for ti in range(T_GRID):
        sh = x[ti * T_SH:(ti + 1) * T_SH]                 # [2048, 4096]
        xts.append(sh.T.astype(f8e5, order="C").reshape(NK, P, T_SH))
    wts = []
    for oj in range(O_GRID):
        sh = weight[oj * O_SH:(oj + 1) * O_SH]
        wts.append(sh.T.astype(bf16, order="C").reshape(NK, P, O_SH))
    in_maps = []
    for core in range(8):
        ti, oj = core // O_GRID, core % O_GRID
        in_maps.append({"x": xts[ti], "w": wts[oj]})
    return in_maps


def kernel(x, weight):
    in_maps = make_in_maps(x, weight)
    nc = _get_nc()
    res = run_bass_kernel_spmd(nc, in_maps, list(range(8)))
    out = np.empty((T_FULL, O_FULL), dtype=np.float32)
    for core in range(8):
        ti, oj = core // O_GRID, core % O_GRID
        out[ti * T_SH:(ti + 1) * T_SH, oj * O_SH:(oj + 1) * O_SH] = (
            np.asarray(res.results[core]["out"]).T.astype(np.float32)
        )
    return out.reshape(B, S, O_FULL)


# revision 6
# speedup vs baseline: 1.0030x; 1.0030x over previous
"""BiRealLinear Trainium2 kernel (w-stationary fp8 DoubleRow, f16 out).

Computes out = binact(x) @ quant_weight(w).T for
  x [4, 2048, 4096] f32, w [4096, 4096] f32  ->  out [4, 2048, 4096] f32

Forward semantics (STE parts drop out in forward):
  binact(x)       = sign(x)                      in {-1, 0, +1}
  quant_weight(w) = mean(|w|, axis=1) * sign(w)  per-output-row scale

So out[t, o] = scale[o] * sum_i sign(x[t,i]) * sign(w[o,i]).

8 cores in a 4 (token) x 2 (out-feature) grid; each core does a
[2048 x 4096 x 2048] sign-matmul. Host supplies both operands
transposed (contraction dim i outermost): w as bf16 k-tiles (bf16 is
sign-exact and loses <1e-4 on mean|w|), x as fp8e5 k-tiles REINTERPRETED
as fp8e4 bytes (the sign bit survives reinterpretation; magnitudes are
irrelevant because the first device op is sign()). On-device per core:
  - w DMAs on the sync queue, x + output DMAs on the gpsimd queue,
  - ACT sign -> fp8e4 (+/-1 exact); x is signed IN PLACE pair-wide,
  - |w| = w * sign(w) (DVE mult) accumulated into 2 f16 running sums,
    one in-place add + 16 one-column PE matmuls against ones -> scale
    as a COLUMN [128, 16] (o on partitions),
  - PE matmul fp8 DoubleRow, W STATIONARY: out psum = [128 o, 512 t],
    so the per-o scale is a per-partition ACT scalar downstream,
  - token half 0 runs DURING the loads, k split in chunks of [4, 4, 8]
    k-pairs (an in-order PE only trickles until a unit's last k-pair
    lands, so early chunks are short), with an exact f16 SBUF
    accumulator across chunks (integer sums <= 2048 per chunk);
    chunk-1/2 tile loads are zipped unit-by-unit into the chunk-0/1
    streams so every engine queue stays in readiness order,
  - token half 1 runs after all tiles are resident as ONE 8-k-pair x2
    chunk... no: one 16-k-pair psum group per unit, drained DIRECTLY by
    ACT: out_f16 = Copy(psum * scale_col) -> DMA. No DVE work at all.
Output DRAM is [o, t] f16 per core; host transposes + upcasts to f32.
"""

import sys

import numpy as np

try:
    import concourse.bacc as bacc  # noqa: F401
except ImportError:
    sys.path.insert(0, "/opt/trn_rl_repo")

import ml_dtypes

import concourse.bacc as bacc
import concourse.mybir as mybir
import concourse.tile as tile
from concourse.bass_utils import run_bass_kernel_spmd

dt = mybir.dt

# ---- problem geometry (hardcoded; full input is [8192, 4096] x [4096, 4096])
B, S, I_FULL, O_FULL = 4, 2048, 4096, 4096
T_FULL = B * S                      # 8192 tokens
T_GRID, O_GRID = 4, 2               # core grid: 4 token shards x 2 out shards
T_SH = T_FULL // T_GRID             # 2048 tokens per core
O_SH = O_FULL // O_GRID             # 2048 out features per core

P = 128                             # partitions
NK = I_FULL // P                    # 32 k-tiles
NKP = NK // 2                       # 16 k-pairs (DoubleRow contracts 2 tiles)
CH0 = [(0, 4), (4, 4), (8, 8)]      # th0 chunks: (k-pair base, count)
NTH = 2                             # token halves
TH = T_SH // NTH                    # 1024 tokens per half
NTP = TH // 512                     # 2 token panels (512) per half
NOB = O_SH // P                     # 16 o-blocks of 128
NWA = 2                             # |w| k-reduce accumulators
NU = NOB * NTP                      # 32 units per (th, chunk)


def build_nc():
    nc = bacc.Bacc("TRN2", target_bir_lowering=False, debug=False, num_devices=8)
    x = nc.dram_tensor("x", [NK, P, T_SH], dt.float8e4, kind="ExternalInput")
    w = nc.dram_tensor("w", [NK, P, O_SH], dt.bfloat16, kind="ExternalInput")
    out = nc.dram_tensor("out", [O_SH, T_SH], dt.float16, kind="ExternalOutput")

    with tile.TileContext(nc) as tc:
        with (
            tc.tile_pool(name="single", bufs=1) as sb,
            tc.tile_pool(name="wstg", bufs=3) as wstg,
            tc.tile_pool(name="wab", bufs=2) as wabp,
            tc.tile_pool(name="wacc", bufs=NWA) as waccp,
            tc.tile_pool(name="wp", bufs=NKP) as wpp,
            tc.tile_pool(name="xp", bufs=2 * NKP) as xpp,
            tc.tile_pool(name="acc", bufs=NU + 2) as accp,
            tc.tile_pool(name="td", bufs=3) as tdp,
            tc.tile_pool(name="og", bufs=4) as ogp,
            tc.tile_pool(name="ps", bufs=7, space="PSUM") as psp,
            tc.tile_pool(name="pss", bufs=1, space="PSUM") as pssp,
        ):
            ones_col = sb.tile([P, 1], dt.float16)      # k-reduce ones
            nc.vector.memset(ones_col[:], 1.0)
            s_cols = sb.tile([P, NOB], dt.float32)      # scale columns
            pscol = pssp.tile([P, NOB], dt.float32, tag="pss", name="pscol")

            WP = [None] * NKP                 # fp8 w pair-tiles, all resident
            XP = {}                           # (th, kp) -> fp8 x pair-tile
            WACC = []                         # f16 |w| accumulators
            for a in range(NWA):
                wa = waccp.tile([P, O_SH], dt.float16, tag="wacc",
                                name=f"wacc_{a}")
                nc.vector.memset(wa[:], 0.0)
                WACC.append(wa)

            def load_w_pair(kp):
                wp_t = wpp.tile([P, 2, O_SH], dt.float8e4, tag="wp",
                                name=f"wp_{kp}")
                for j in range(2):
                    k = 2 * kp + j
                    wtb = wstg.tile([P, O_SH], dt.bfloat16, tag="wstg",
                                    name=f"wtb_{k}")
                    nc.sync.dma_start(wtb[:], w[k])
                    nc.scalar.sign(wp_t[:, j, :], wtb[:])
                    wab = wabp.tile([P, O_SH], dt.float16, tag="wab",
                                    name=f"wab_{k}")
                    nc.vector.tensor_mul(wab[:], wtb[:], wp_t[:, j, :])
                    nc.vector.tensor_add(WACC[k % NWA][:], wab[:],
                                         WACC[k % NWA][:])
                WP[kp] = wp_t

            def load_x_pair(th, kp):
                # x bytes land straight in the pair tile; sign runs in place
                # (fp8e5 bytes viewed as fp8e4: the sign bit is unchanged).
                xp_t = xpp.tile([P, 2, TH], dt.float8e4, tag="xp",
                                name=f"xp_{th}_{kp}")
                for j in range(2):
                    k = 2 * kp + j
                    nc.gpsimd.dma_start(xp_t[:, j, :],
                                        x[k][:, th * TH:(th + 1) * TH])
                nc.scalar.sign(xp_t[:], xp_t[:])
                XP[(th, kp)] = xp_t

            def compute_scale():
                nc.vector.tensor_add(WACC[0][:], WACC[1][:], WACC[0][:])
                for ob in range(NOB):
                    nc.tensor.matmul(
                        pscol[:, ob:ob + 1],
                        lhsT=WACC[0][:, ob * P:(ob + 1) * P],
                        rhs=ones_col[:],
                        start=True, stop=True,
                        skip_group_check=True,
                    )
                nc.vector.tensor_scalar_mul(s_cols[:], pscol[:], 1.0 / I_FULL)

            def store_unit(og, ob, pan):
                nc.gpsimd.dma_start(
                    out[ob * P:(ob + 1) * P, pan * 512:(pan + 1) * 512],
                    og[:])

            def mm_unit_th0(ci, ob, tp, accs, pre=None):
                base, n = CH0[ci]
                first, last = (ci == 0), (ci == len(CH0) - 1)
                u = ob * NTP + tp
                pst = psp.tile([P, 512], dt.float32, tag="ps",
                               name=f"ps_0_{ci}_{u}")
                for kpl in range(n):
                    if pre is not None and kpl == 0:
                        pre()
                    kp = base + kpl
                    nc.tensor.matmul(
                        pst[:],
                        lhsT=WP[kp][:, :, ob * P:(ob + 1) * P],
                        rhs=XP[(0, kp)][:, :, tp * 512:(tp + 1) * 512],
                        start=(kpl == 0), stop=(kpl == n - 1),
                        perf_mode=mybir.MatmulPerfMode.DoubleRow,
                    )
                if first:
                    acc = accp.tile([P, 512], dt.float16, tag="acc",
                                    name=f"acc_{u}")
                    accs[u] = acc
                    nc.vector.tensor_copy(acc[:], pst[:])
                elif not last:
                    nc.vector.tensor_add(accs[u][:], pst[:], accs[u][:])
                else:
                    td = tdp.tile([P, 512], dt.float16, tag="td",
                                  name=f"td_{u}")
                    nc.vector.tensor_add(td[:], pst[:], accs[u][:])
                    og = ogp.tile([P, 512], dt.float16, tag="og",
                                  name=f"og_0_{u}")
                    nc.scalar.activation(
                        og[:], td[:],
                        mybir.ActivationFunctionType.Copy,
                        scale=s_cols[:, ob:ob + 1])
                    store_unit(og, ob, tp)

            def mm_unit_th1(ob, tp, pre=None):
                u = ob * NTP + tp
                pst = psp.tile([P, 512], dt.float32, tag="ps",
                               name=f"ps_1_{u}")
                for kp in range(NKP):
                    if pre is not None and kp == 0:
                        pre()
                    nc.tensor.matmul(
                        pst[:],
                        lhsT=WP[kp][:, :, ob * P:(ob + 1) * P],
                        rhs=XP[(1, kp)][:, :, tp * 512:(tp + 1) * 512],
                        start=(kp == 0), stop=(kp == NKP - 1),
                        perf_mode=mybir.MatmulPerfMode.DoubleRow,
                    )
                og = ogp.tile([P, 512], dt.float16, tag="og",
                              name=f"og_1_{u}")
                nc.scalar.activation(
                    og[:], pst[:],
                    mybir.ActivationFunctionType.Copy,
                    scale=s_cols[:, ob:ob + 1])
                store_unit(og, ob, NTP + tp)

            units = [(ob, tp) for ob in range(NOB) for tp in range(NTP)]
            accs = [None] * NU

            # ---- program order: loads/signs feed the PE just ahead of use;
            # later-chunk loads zip with earlier-chunk units so each engine
            # queue stays in readiness order.
            for kp in range(4):               # chunk 0 tiles
                load_w_pair(kp)
                load_x_pair(0, kp)
            for i, (ob, tp) in enumerate(units):      # chunk 0 + chunk-1 loads
                pre = None
                if i % 8 == 0:
                    kp1 = 4 + i // 8
                    def pre(kp1=kp1):
                        load_w_pair(kp1)
                        load_x_pair(0, kp1)
                mm_unit_th0(0, ob, tp, accs, pre)
            for i, (ob, tp) in enumerate(units):      # chunk 1 + chunk-2 loads
                pre = None
                if i % 4 == 0:
                    kp1 = 8 + i // 4
                    def pre(kp1=kp1):
                        load_w_pair(kp1)
                        load_x_pair(0, kp1)
                mm_unit_th0(1, ob, tp, accs, pre)
            compute_scale()
            for kp in range(8):               # x th1 front half
                load_x_pair(1, kp)
            for i, (ob, tp) in enumerate(units):      # chunk 2 + th1-x loads
                pre = None
                if i % 4 == 0:
                    kp1 = 8 + i // 4
                    def pre(kp1=kp1):
                        load_x_pair(1, kp1)
                mm_unit_th0(2, ob, tp, accs, pre)
            for ob, tp in units:                      # th1, fully resident
                mm_unit_th1(ob, tp)

    nc.compile()
    return nc


_NC_CACHE = None


def _get_nc():
    global _NC_CACHE
    if _NC_CACHE is None:
        _NC_CACHE = build_nc()
    return _NC_CACHE


def make_in_maps(x, weight):
    """Host-side shard + layout prep: per-core transposed k-tiles."""
    bf16 = ml_dtypes.bfloat16
    f8e5 = ml_dtypes.float8_e5m2
    f8e4 = ml_dtypes.float8_e4m3
    x = np.asarray(x, dtype=np.float32).reshape(T_FULL, I_FULL)
    weight = np.asarray(weight, dtype=np.float32)
    xts = []
    for ti in range(T_GRID):
        sh = x[ti * T_SH:(ti + 1) * T_SH]                 # [2048, 4096]
        xt = sh.T.astype(f8e5, order="C").view(f8e4)
        xts.append(xt.reshape(NK, P, T_SH))
    wts = []
    for oj in range(O_GRID):
        sh = weight[oj * O_SH:(oj + 1) * O_SH]
        wts.append(sh.T.astype(bf16, order="C").reshape(NK, P, O_SH))
    in_maps = []
    for core in range(8):
        ti, oj = core // O_GRID, core % O_GRID
        in_maps.append({"x": xts[ti], "w": wts[oj]})
    return in_maps


def kernel(x, weight):
    in_maps = make_in_maps(x, weight)
    nc = _get_nc()
    res = run_bass_kernel_spmd(nc, in_maps, list(range(8)))
    out = np.empty((T_FULL, O_FULL), dtype=np.float32)
    for core in range(8):
        ti, oj = core // O_GRID, core % O_GRID
        out[ti * T_SH:(ti + 1) * T_SH, oj * O_SH:(oj + 1) * O_SH] = (
            np.asarray(res.results[core]["out"]).T.astype(np.float32)
        )
    return out.reshape(B, S, O_FULL)
